# revision 56
# baseline (speedup 1.0000x reference)
"""AttentionRNNCell Trainium2 kernel (v2: dual-layout fp8, no on-chip transposes).

Math (per batch row b):
  et[t]  = V_a . tanh( (h W_a + b_a) + x[t] U_a )        t in [0, TE)
  at     = exp(et);  s = sum(at)
  ctx    = (sum_t at[t] x[t]) / s
  zt     = sigmoid(h W_z + [inp, ctx] C_z + b_z)
  rt     = sigmoid(h W_r + [inp, ctx] C_r + b_r)
  tht    = tanh((rt*h) U_p + [inp, ctx] C_p + b_p)
  ht     = (1-zt)*h + zt*tht

Distribution: data-parallel over batch B=128 across 8 cores (16 rows each).

Design notes (per core):
  - The host ships x twice in fp8e4m3 (8.4MB each = 16.8MB total, same HBM
    bytes as one bf16 copy): xt = x^T in DoubleRow pair layout [p, j, t]
    (e = j*128+p) for the attention matmul, and xn natural [p, tc, e]
    (t = tc*128+p) for the context matmul. No on-chip transposes (the v1
    baseline spent ~650us in 512 xbar-transpose DMAs).
  - Attention matmul: U_a stationary [128,2,128] fp8 DoubleRow -> K=256 in
    one pass; out ux[u, t-chunk] in PSUM; ACT tanh (bias = (hW_a+b_a)^T col)
    writes fp8 in t'-interleaved order t' = tp*tc_n + tc so that:
  - V-dot: V_a stationary (M=1) streams tanh -> et row [1, te] in t' order;
    DVE copies rows to bf16 SBUF; one small DMA per b redistributes
    [1, te] -> [128, tc_n] (32B contiguous runs per partition).
  - exp on [128, tc_n] columns (accum_out -> partition-partial sums);
    s_b via ones-matmul; 1/s folded into the ctx-row copy (tensor_scalar).
  - Context matmul: at (fp8, DoubleRow pair layout) stationary, xn streams.
  - Gates in bf16, transposed orientation; sigmoid computed as
    0.5*tanh(0.5x)+0.5 to stay on the exp/tanh ACT table set.
"""

from contextlib import ExitStack

import numpy as np
import ml_dtypes

import concourse.bass as bass
import concourse.mybir as mybir
import concourse.tile as tile

F8NP = ml_dtypes.float8_e4m3
BF16 = ml_dtypes.bfloat16
F32 = mybir.dt.float32
BF = mybir.dt.bfloat16
F8 = mybir.dt.float8e4
AF = mybir.ActivationFunctionType
DR = mybir.MatmulPerfMode.DoubleRow

B, TE, U, IN_DIM = 128, 2048, 256, 256
N_CORES = 8
BS = B // N_CORES  # 16 batch rows per core
P = 128
UC = U // P  # u-chunks (2)


def split_multi_waits(nc, max_waits=1):
    """This container's walrus rejects instructions carrying more than one
    sync wait. Hoist extra waits onto standalone same-engine NoOps inserted
    immediately before the offending instruction (semantically identical:
    the engine blocks on each wait in order before executing it)."""
    n_new = 0
    for f in nc.m.functions:
        for blk in f.blocks:
            new_insts = []
            for inst in blk.instructions:
                si = inst.sync_info
                waits = list(si.on_wait) if si and si.on_wait else []
                if len(waits) > max_waits:
                    for w in waits[:-max_waits]:
                        nop = mybir.InstNoOp(
                            name=f"{inst.name}-hw{n_new}", ins=[], outs=[]
                        )
                        nop.engine = inst.engine
                        nop.sync_info = mybir.SyncInfo(on_wait=[w], on_update=[])
                        new_insts.append(nop)
                        n_new += 1
                    si.on_wait = waits[-max_waits:]
                new_insts.append(inst)
            blk.instructions = new_insts
    return n_new


def build_nc(bs=BS, te=TE, dr_main=True, dr_vdot=True, dr_ctx=True,
             swi_main=False, pair16=True, split_waits=True, debug=False):
    """Build the per-core Bass module. Parametrized so a small variant can be
    simulated quickly; the production shape is (bs=16, te=2048)."""
    tc_n = te // P           # 128-row t-chunks (16)
    nq = te // 512           # 512-wide et/vdot chunks (4)
    ux_w = min(1024, te)     # tanh tile width (PSUM: 2 banks)
    n_ux = te // ux_w        # tanh tiles per (b, uc)
    tcq = ux_w // P          # t-chunks per tanh tile (8)

    nc = bass.Bass()
    # xt[p, s, j] = x[t(s), j*128+p] with stream order t(s) = (s%tc_n)*128 + s//tc_n
    # (t' column order: ux/tanh/et come out so the per-b DRAM gather is clean;
    #  DoubleRow pair dim j innermost so the PE streams 2 adjacent fp8/cycle)
    if pair16:
        # DoubleRow pair at 16B stride: xt[p, g, j, ti] = x[t(g*16+ti), j*128+p]
        xt_d = nc.declare_dram_parameter("xt", [bs, P, te // 16, 2, 16], F8, isOutput=False)
        # xn[p, c, g, j, ei] = x[(2c+j)*128+p, g*16+ei]
        xn_d = nc.declare_dram_parameter(
            "xn", [bs, P, tc_n // 2, U // 16, 2, 16], F8, isOutput=False)
    else:
        xt_d = nc.declare_dram_parameter("xt", [bs, P, te, 2], F8, isOutput=False)
        # xn[p, c, e, j] = x[(2c+j)*128+p, e] (pair dim innermost)
        xn_d = nc.declare_dram_parameter("xn", [bs, P, tc_n // 2, U, 2], F8, isOutput=False)
    ua_d = nc.declare_dram_parameter("ua", [P, 2, U], F8, isOutput=False)
    # SwInterleave weight layout: [p, uc, 2*i+j] = U_a[j*128+p, uc*128 + (127-i)]
    uasw_d = nc.declare_dram_parameter("uasw", [P, UC, U], F8, isOutput=False)
    va_d = nc.declare_dram_parameter("va", [P, 2], BF, isOutput=False)
    wxpbT_d = nc.declare_dram_parameter("wxpbT", [P, UC, bs], F32, isOutput=False)
    hT_d = nc.declare_dram_parameter("hT", [P, UC, bs], F32, isOutput=False)
    g0T_d = nc.declare_dram_parameter("g0T", [3, P, UC, bs], F32, isOutput=False)
    cz_d = nc.declare_dram_parameter("cz", [P, UC, U], BF, isOutput=False)
    cr_d = nc.declare_dram_parameter("cr", [P, UC, U], BF, isOutput=False)
    cp_d = nc.declare_dram_parameter("cp", [P, UC, U], BF, isOutput=False)
    up_d = nc.declare_dram_parameter("up", [P, UC, U], BF, isOutput=False)
    id_d = nc.declare_dram_parameter("ident", [P, P], F32, isOutput=False)
    ht_d = nc.declare_dram_parameter("ht", [bs, U], F32, isOutput=True)
    # DRAM scratch for free-dim -> partition redistributions (SBUF APs cannot
    # express those; DRAM APs can be reshaped arbitrarily)
    etr_d = nc.declare_dram_parameter("etr_scratch", [bs, te], BF, isOutput=True)
    ctxr_d = nc.declare_dram_parameter("ctxr_scratch", [bs, U], BF, isOutput=True)
    if debug:
        dbg_tanh_d = nc.declare_dram_parameter("dbg_tanh", [P, UC, te], F8, isOutput=True)
        dbg_etrow_d = nc.declare_dram_parameter("dbg_etrow", [bs, te], BF, isOutput=True)
        dbg_etT_d = nc.declare_dram_parameter("dbg_etT", [bs, P, te // P], BF, isOutput=True)
        dbg_at_d = nc.declare_dram_parameter("dbg_at", [bs, P, 2, 16], F8, isOutput=True)
        dbg_spart_d = nc.declare_dram_parameter("dbg_spart", [P, bs], F32, isOutput=True)
        dbg_recip_d = nc.declare_dram_parameter("dbg_recip", [1, bs], F32, isOutput=True)
        dbg_ctxrows_d = nc.declare_dram_parameter("dbg_ctxrows", [bs, U], BF, isOutput=True)
        dbg_xn_d = nc.declare_dram_parameter("dbg_xn", [P, te // P // 2, U, 2], F8, isOutput=True)
        dbg_cps_d = nc.declare_dram_parameter("dbg_cps", [bs, U], F32, isOutput=True)

    with tile.TileContext(nc) as tc, ExitStack() as ctx:
        singles = ctx.enter_context(tc.tile_pool(name="singles", bufs=1))
        xt_p = ctx.enter_context(tc.tile_pool(name="xt", bufs=4))
        xn_p = ctx.enter_context(tc.tile_pool(name="xn", bufs=6))
        tanh_p = ctx.enter_context(tc.tile_pool(name="tanh", bufs=3))
        etrow_p = ctx.enter_context(tc.tile_pool(name="etrow", bufs=4))
        etT_p = ctx.enter_context(tc.tile_pool(name="etT", bufs=4))
        at_p = ctx.enter_context(tc.tile_pool(name="at", bufs=4))
        small_p = ctx.enter_context(tc.tile_pool(name="small", bufs=4))
        ux_ps = ctx.enter_context(tc.tile_pool(name="uxps", bufs=2, space="PSUM"))
        et_ps = ctx.enter_context(tc.tile_pool(name="etps", bufs=2, space="PSUM"))
        ctx_ps = ctx.enter_context(tc.tile_pool(name="ctxps", bufs=2, space="PSUM"))

        # first-matmul weights (tiny) first, then the first row's xt
        ua_sb = singles.tile([P, 2, U], F8)
        if swi_main:
            nc.sync.dma_start(out=ua_sb, in_=uasw_d[:, :, :])
        else:
            nc.sync.dma_start(out=ua_sb, in_=ua_d[:, :, :])
        wxpb_pre = singles.tile([P, UC, bs], F32, name="wxpb_sb")
        nc.scalar.dma_start(out=wxpb_pre, in_=wxpbT_d[:, :, :])
        xt_shape0 = [P, te // 16, 2, 16] if pair16 else [P, te, 2]
        xt_first = xt_p.tile(xt_shape0, F8, name="xtpre0")
        h0 = xt_shape0[1] // 2
        nc.sync.dma_start(out=xt_first[:, 0:h0], in_=xt_d[0, :, 0:h0])
        nc.scalar.dma_start(out=xt_first[:, h0:], in_=xt_d[0, :, h0:])

        # prefetch the first batch rows' x
        xt_shape = [P, te // 16, 2, 16] if pair16 else [P, te, 2]
        xn_shape = ([P, tc_n // 2, U // 16, 2, 16] if pair16 else
                    [P, tc_n // 2, U, 2])
        xt_pre, xn_pre = {}, {}
        xt_pre[0] = xt_first
        for b in range(1, min(2, bs)):
            xt_pre[b] = xt_p.tile(xt_shape, F8, name=f"xtpre{b}")
            nc.sync.dma_start(out=xt_pre[b], in_=xt_d[b])
        for b in range(min(2, bs)):
            xn_pre[b] = xn_p.tile(xn_shape, F8, name=f"xnpre{b}")
            nc.scalar.dma_start(out=xn_pre[b], in_=xn_d[b])
        va_sb = singles.tile([P, 2], BF)
        nc.scalar.dma_start(out=va_sb, in_=va_d[:, :])
        wxpb_sb = wxpb_pre
        hT_sb = singles.tile([P, UC, bs], F32)
        nc.scalar.dma_start(out=hT_sb, in_=hT_d[:, :, :])
        g0_sb = singles.tile([P, 3, UC, bs], F32)
        nc.scalar.dma_start(out=g0_sb, in_=g0T_d[:, :, :, :].rearrange("g p c b -> p g c b"))
        gate_w = {}
        for name, d in (("cz", cz_d), ("cr", cr_d), ("cp", cp_d), ("up", up_d)):
            w_sb = singles.tile([P, UC, U], BF, name=f"{name}_sb")
            nc.scalar.dma_start(out=w_sb, in_=d[:, :, :])
            gate_w[name] = w_sb
        id_sb = singles.tile([P, P], F32)
        nc.scalar.dma_start(out=id_sb, in_=id_d[:, :])
        ones_sb = singles.tile([P, 1], F32)
        nc.vector.memset(ones_sb, 1.0)
        ones_row = singles.tile([1, P], F32)
        nc.vector.memset(ones_row, 1.0)
        s_part = singles.tile([P, bs], F32)
        recip_row = singles.tile([1, 16], F32)
        nc.vector.memset(recip_row, 1.0)
        ctx_rows = singles.tile([1, bs, U], BF)

        def finish_row(b, etT, xn_sb):
            # exp -> at (fp8, DoubleRow pair layout [p, j, tc2]); partial sums
            at8 = at_p.tile([P, 2, 16], F8)
            nc.scalar.activation(
                out=at8[:, :, 0 : tc_n // 2].rearrange("p j c -> p c j"),
                in_=etT,
                func=AF.Exp,
                accum_out=s_part[:, b : b + 1],
            )

            # s_b = sum over partitions; 1/s applied after the ctx transpose
            # (lives in the ctx pool: in the et pool its slow consumer -- the
            # DVE reciprocal -- held a buffer the next vdot needed)
            s_ps = ctx_ps.tile([1, 1], F32, tag="ctxps", name="s_ps")
            nc.tensor.matmul(out=s_ps, lhsT=ones_sb, rhs=s_part[:, b : b + 1])
            nc.vector.reciprocal(recip_row[:, b : b + 1], s_ps)

            # context matmul: at stationary, xn streams; ctx row [1, U]
            cps = ctx_ps.tile([1, U], F32, tag="ctxps")
            if dr_ctx:
                for j in range(tc_n // 2):
                    nc.tensor.matmul(
                        out=cps,
                        lhsT=at8[:, :, j : j + 1],
                        rhs=(xn_sb[:, j, :, :, :].rearrange("p g k e -> p k g e")
                             if pair16 else
                             xn_sb[:, j, :, :].rearrange("p e j -> p j e")),
                        perf_mode=DR,
                        start=(j == 0),
                        stop=(j == tc_n // 2 - 1),
                    )
            else:
                for tcc in range(tc_n):
                    nc.tensor.matmul(
                        out=cps,
                        lhsT=at8[:, tcc % 2, tcc // 2 : tcc // 2 + 1],
                        rhs=(xn_sb[:, tcc // 2, :, tcc % 2, :]
                             if pair16 else xn_sb[:, tcc // 2, :, tcc % 2]),
                        start=(tcc == 0),
                        stop=(tcc == tc_n - 1),
                    )
            nc.vector.tensor_copy(ctx_rows[:, b, :], cps)
            nc.sync.dma_start(out=ctxr_d[b : b + 1, :], in_=ctx_rows[:, b, :])
            if debug:
                cps_tap = small_p.tile([1, U], F32, tag="cpstap")
                nc.vector.tensor_copy(cps_tap, cps)
                nc.sync.dma_start(out=dbg_cps_d[b : b + 1, :], in_=cps_tap)
                nc.sync.dma_start(out=dbg_at_d[b, :, :, :], in_=at8)

        warm_rhs = singles.tile([P, 512], BF, name="warm_rhs")
        nc.vector.memset(warm_rhs, 0.0)
        ones_bf = singles.tile([P, 1], BF, name="ones_bf")
        nc.vector.memset(ones_bf, 1.0)
        n_warm = 12 if te >= 2048 else 2
        for w in range(n_warm):
            wps = et_ps.tile([1, 512], F32, tag="etps", name=f"warm{w}")
            nc.tensor.matmul(out=wps, lhsT=ones_bf, rhs=warm_rhs)

        def mm_half(b, uc, xt_sb, tanh_t):
            """Attention matmul for one u-chunk over the full t range, in
            [128, ux_w] PSUM tiles each followed by one tanh."""
            for c in range(n_ux):
                ux = ux_ps.tile([P, ux_w], F32, tag="uxps", name=f"ux{uc}_{c}")
                for m0 in range(0, ux_w, 512):
                    t0 = c * ux_w + m0
                    if pair16:
                        rhs_mm = xt_sb[:, t0 // 16 : t0 // 16 + 32, :, :].rearrange(
                            "p g j t -> p j g t")
                    else:
                        rhs_mm = xt_sb[:, t0 : t0 + 512, :].rearrange("p t j -> p j t")
                    if swi_main:
                        nc.tensor.matmul(
                            out=ux[:, m0 : m0 + 512],
                            lhsT=ua_sb[:, uc, :],
                            rhs=rhs_mm,
                            perf_mode=mybir.MatmulPerfMode.DoubleRowSwInterleave,
                        )
                    elif dr_main:
                        nc.tensor.matmul(
                            out=ux[:, m0 : m0 + 512],
                            lhsT=ua_sb[:, :, uc * P : (uc + 1) * P],
                            rhs=rhs_mm,
                            perf_mode=DR,
                        )
                    else:
                        for j in range(2):
                            nc.tensor.matmul(
                                out=ux[:, m0 : m0 + 512],
                                lhsT=ua_sb[:, j, uc * P : (uc + 1) * P],
                                rhs=(xt_sb[:, t0 // 16 : t0 // 16 + 32, j, :]
                                     if pair16 else xt_sb[:, t0 : t0 + 512, j]),
                                start=(j == 0),
                                stop=(j == 1),
                            )
                nc.scalar.activation(
                    out=tanh_t[:, uc, c * ux_w : (c + 1) * ux_w],
                    in_=ux,
                    func=AF.Tanh,
                    bias=wxpb_sb[:, uc, b : b + 1],
                )

        def vdot_row(b, tanh_t, xn_sb):
            """V-dot -> et row (t' order), DVE copy to bf16, DRAM round trip."""
            et_row = etrow_p.tile([1, te], BF)
            for q in range(nq):
                etq = et_ps.tile([1, 512], F32, tag="etps", name=f"etq{q}")
                for uc in range(UC):
                    nc.tensor.matmul(
                        out=etq,
                        lhsT=va_sb[:, uc : uc + 1],
                        rhs=tanh_t[:, uc, q * 512 : (q + 1) * 512],
                        start=(uc == 0),
                        stop=(uc == UC - 1),
                    )
                nc.vector.tensor_copy(et_row[:, q * 512 : (q + 1) * 512], etq)
            nc.sync.dma_start(out=etr_d[b : b + 1, :], in_=et_row)
            etT = etT_p.tile([P, tc_n], BF)
            nc.sync.dma_start(
                out=etT, in_=etr_d[b, :].rearrange("(tp tc) -> tp tc", tc=tc_n)
            )
            if debug:
                if b == 0:
                    nc.sync.dma_start(out=dbg_tanh_d[:, :, :], in_=tanh_t)
                    if not pair16:
                        nc.sync.dma_start(out=dbg_xn_d[:, :, :], in_=xn_sb)
                nc.sync.dma_start(out=dbg_etrow_d[b : b + 1, :], in_=et_row)
                nc.sync.dma_start(out=dbg_etT_d[b, :, :], in_=etT)
            return etT

        # ---- streaming loop, 2-deep software pipeline ----
        # iter i: load(i), mm(uc0,i), vdot(i-1), mm(uc1,i), exp/s/ctx(i-2).
        # The in-order PE never waits: vdot(i-1) fills the tanh(uc0,i) window,
        # and the et DRAM round trip for row i-1 has a full iteration to land
        # before exp(i-1) runs in iter i+1.
        loaded = dict(xt_pre)
        for b in xt_pre:
            loaded[b] = (xt_pre[b], xn_pre[b])
        tiles = {}
        pend = []
        for i in range(bs + 2):
            if i + 2 < bs and (i + 2) not in loaded:
                xt_sb = xt_p.tile(xt_shape, F8)  # [e%128, t' order, j]
                nc.sync.dma_start(out=xt_sb, in_=xt_d[i + 2])
                xn_sb = xn_p.tile(xn_shape, F8)
                nc.scalar.dma_start(out=xn_sb, in_=xn_d[i + 2])
                loaded[i + 2] = (xt_sb, xn_sb)
            if i < bs:
                xt_sb, xn_sb = loaded.pop(i)
                tanh_t = tanh_p.tile([P, UC, te], BF)  # [u%128, uc, t']
                tiles[i] = (xt_sb, xn_sb, tanh_t)
                mm_half(i, 0, xt_sb, tanh_t)
            if 0 <= i - 1 < bs:
                bp = i - 1
                etT = vdot_row(bp, tiles[bp][2], tiles[bp][1])
                pend.append((bp, etT, tiles[bp][1]))
            if i < bs:
                mm_half(i, 1, xt_sb, tanh_t)
            if pend and pend[0][0] == i - 2:
                finish_row(*pend.pop(0))

        if debug:
            nc.sync.dma_start(out=dbg_spart_d[:, :], in_=s_part)
            nc.sync.dma_start(out=dbg_recip_d[:, :], in_=recip_row)
            nc.sync.dma_start(out=dbg_ctxrows_d[:, :], in_=ctx_rows[0, :, :])

        # ---- tail: transpose ctx rows (straight from DRAM), gates ----
        ctxTu = small_p.tile([P, UC, 16], BF, name="ctxTu")
        if bs < 16:
            nc.vector.memset(ctxTu, 0.0)
        for e in range(UC):
            nc.sync.dma_start_transpose(
                out=ctxTu[:, e, 0:bs], in_=ctxr_d[:, e * P : (e + 1) * P]
            )
        # normalize: ctxT = ctxT_unnorm * (1/s), broadcast 1/s to all partitions
        bc_ps = et_ps.tile([P, 16], F32, tag="etps", name="bc_ps")
        nc.tensor.matmul(out=bc_ps, lhsT=ones_row, rhs=recip_row)
        recip128 = small_p.tile([P, 16], F32, name="recip128")
        nc.vector.tensor_copy(recip128, bc_ps)
        ctxT = small_p.tile([P, UC, 16], BF, name="ctxT")
        for e in range(UC):
            nc.vector.tensor_mul(ctxT[:, e, :], ctxTu[:, e, :], recip128)

        def gate_psum(w_names_rhs, name):
            """psum[uc] = sum over (w, rhs) pairs of w^T @ rhs, per u-chunk."""
            outs = []
            for uc in range(UC):
                g = et_ps.tile([P, bs], F32, tag="etps", name=f"{name}{uc}")
                n_mm = sum(UC for _ in w_names_rhs)
                i = 0
                for w_sb, rhs_fn in w_names_rhs:
                    for e in range(UC):
                        nc.tensor.matmul(
                            out=g,
                            lhsT=w_sb[:, e, uc * P : (uc + 1) * P],
                            rhs=rhs_fn(e),
                            start=(i == 0),
                            stop=(i == n_mm - 1),
                        )
                        i += 1
                outs.append(g)
            return outs

        # zt^T, rt^T = sigmoid(g0 + C_*^T ctx^T) ; sigmoid via tanh:
        # sigmoid(x) = 0.5*tanh(0.5x) + 0.5 (stays on the exp/tanh table set)
        zt_sb = small_p.tile([P, UC, bs], F32)
        rt_sb = small_p.tile([P, UC, bs], F32)
        for gi, (wname, dst) in enumerate((("cz", zt_sb), ("cr", rt_sb))):
            gps = gate_psum([(gate_w[wname], lambda e: ctxT[:, e, 0:bs])], wname)
            for uc in range(UC):
                tmp = small_p.tile([P, bs], F32, tag="gtmp", name=f"t{wname}{uc}")
                nc.vector.tensor_add(tmp, gps[uc], g0_sb[:, gi, uc, :])
                th = small_p.tile([P, bs], F32, tag="gtmp", name=f"th{wname}{uc}")
                nc.scalar.activation(out=th, in_=tmp, func=AF.Tanh, scale=0.5)
                nc.vector.tensor_scalar(
                    dst[:, uc, :], th, 0.5, 0.5,
                    mybir.AluOpType.mult, mybir.AluOpType.add,
                )

        # rh^T = rt^T * h^T ; tht^T = tanh(g0p + U_p^T rh^T + C_p^T ctx^T)
        rh_sb = small_p.tile([P, UC, bs], BF)
        for uc in range(UC):
            nc.vector.tensor_mul(rh_sb[:, uc, :], rt_sb[:, uc, :], hT_sb[:, uc, :])
        gps = gate_psum(
            [(gate_w["up"], lambda e: rh_sb[:, e, :]),
             (gate_w["cp"], lambda e: ctxT[:, e, 0:bs])],
            "cp",
        )
        ht_nat = small_p.tile([bs, U], F32)
        for uc in range(UC):
            tmp = small_p.tile([P, bs], F32, tag="gtmp", name=f"tp{uc}")
            nc.vector.tensor_add(tmp, gps[uc], g0_sb[:, 2, uc, :])
            tht = small_p.tile([P, bs], F32, tag="gtmp", name=f"tht{uc}")
            nc.scalar.activation(out=tht, in_=tmp, func=AF.Tanh)
            # ht^T = h^T + zt^T*(tht^T - h^T)
            nc.vector.tensor_sub(tht, tht, hT_sb[:, uc, :])
            nc.vector.tensor_mul(tht, tht, zt_sb[:, uc, :])
            nc.vector.tensor_add(tht, tht, hT_sb[:, uc, :])
            tp = et_ps.tile([bs, P], F32, tag="etps", name=f"htp{uc}")
            nc.tensor.transpose(tp, tht[:, 0:bs], id_sb)
            nc.vector.tensor_copy(ht_nat[:, uc * P : (uc + 1) * P], tp)
        nc.sync.dma_start(out=ht_d[:, :], in_=ht_nat)

    if split_waits:
        split_multi_waits(nc)
    return nc


def _host_prep(inputs, h_tm, V_a, W_a, U_a, b_a, C_z, W_z, b_z, C_r, W_r, b_r,
               C_p, U_p, b_p):
    """Fold everything not depending on x_seq into small per-core tensors."""
    wxpb = h_tm @ W_a + b_a                                # [B, U]
    g_z0 = h_tm @ W_z + inputs @ C_z[:IN_DIM] + b_z        # [B, U]
    g_r0 = h_tm @ W_r + inputs @ C_r[:IN_DIM] + b_r
    g_p0 = inputs @ C_p[:IN_DIM] + b_p

    def chunkT(a):  # [bs, U] -> [P, UC, bs] with [p, c, b] = a[b, c*128+p]
        return np.ascontiguousarray(
            a.T.reshape(UC, P, -1).transpose(1, 0, 2).astype(np.float32)
        )

    def wprep(w):  # [U, U] -> [P, UC, U] bf16 with [p, c, u] = w[c*128+p, u]
        return np.ascontiguousarray(
            w.reshape(UC, P, U).transpose(1, 0, 2).astype(BF16)
        )

    # SwInterleave weights: [p, uc, 2*i+j] = U_a[j*128+p, uc*128 + (127-i)]
    ua_pj = U_a.reshape(2, P, U).transpose(1, 0, 2)   # [p, j, u]
    uasw = np.zeros((P, UC, U), dtype=np.float32)
    i_idx = np.arange(P)
    for uc_i in range(UC):
        cols = uc_i * P + (P - 1 - i_idx)             # reversed column order
        for j in range(2):
            uasw[:, uc_i, 2 * i_idx + j] = ua_pj[:, j, cols]
    shared = {
        "ua": np.ascontiguousarray(
            U_a.reshape(2, P, U).transpose(1, 0, 2).astype(F8NP)
        ),
        "uasw": np.ascontiguousarray(uasw.astype(F8NP)),
        "va": np.ascontiguousarray(V_a.reshape(2, P).T.astype(BF16)),
        "cz": wprep(C_z[IN_DIM:]),
        "cr": wprep(C_r[IN_DIM:]),
        "cp": wprep(C_p[IN_DIM:]),
        "up": wprep(U_p),
        "ident": np.eye(P, dtype=np.float32),
    }
    per_core = []
    for c in range(N_CORES):
        s = slice(c * BS, (c + 1) * BS)
        per_core.append(
            {
                "wxpbT": chunkT(wxpb[s]),
                "hT": chunkT(h_tm[s]),
                "g0T": np.ascontiguousarray(
                    np.stack([chunkT(g_z0[s]), chunkT(g_r0[s]), chunkT(g_p0[s])])
                ),
                **shared,
            }
        )
    return per_core


def _x_prep(x_seq, pair16=True):
    """Per-core fp8 dual layouts of x (see build_nc layout comments)."""
    x8 = x_seq.astype(F8NP)
    tc_n = TE // P
    out = []
    for c in range(N_CORES):
        xs = x8[c * BS : (c + 1) * BS]                     # [bs, TE, U]
        # xt[b, p, s, j] = x[b, t(s), j*128+p], t(s) = (s%tc_n)*128 + s//tc_n
        xp = xs.reshape(BS, tc_n, P, U).transpose(0, 2, 1, 3).reshape(BS, TE, U)
        xt = np.ascontiguousarray(
            xp.reshape(BS, TE, 2, P).transpose(0, 3, 1, 2)
        )
        # xn[b, p, c, e, j] = x[b, (2c+j)*128+p, e]
        xn = np.ascontiguousarray(
            xs.reshape(BS, tc_n // 2, 2, P, U).transpose(0, 3, 1, 4, 2)
        )
        if pair16:
            xt = np.ascontiguousarray(
                xt.reshape(BS, P, TE // 16, 16, 2).transpose(0, 1, 2, 4, 3)
            )
            xn = np.ascontiguousarray(
                xn.reshape(BS, P, tc_n // 2, U // 16, 16, 2).transpose(0, 1, 2, 3, 5, 4)
            )
        out.append({"xt": xt, "xn": xn})
    return out


def kernel(inputs, h_tm, x_seq, V_a, W_a, U_a, b_a, C_z, W_z, b_z,
           C_r, W_r, b_r, C_p, U_p, b_p):
    from concourse.bass_utils import run_bass_kernel_spmd

    args = {k: np.asarray(v, dtype=np.float32) for k, v in dict(
        inputs=inputs, h_tm=h_tm, V_a=V_a, W_a=W_a, U_a=U_a, b_a=b_a,
        C_z=C_z, W_z=W_z, b_z=b_z, C_r=C_r, W_r=W_r, b_r=b_r,
        C_p=C_p, U_p=U_p, b_p=b_p).items()}
    x_seq = np.asarray(x_seq, dtype=np.float32)

    per_core = _host_prep(**args)
    x_maps = _x_prep(x_seq)
    in_maps = [{**per_core[c], **x_maps[c]} for c in range(N_CORES)]

    nc = build_nc()
    res = run_bass_kernel_spmd(nc, in_maps, core_ids=list(range(N_CORES)))
    return np.concatenate([res.results[c]["ht"] for c in range(N_CORES)], axis=0)


# revision 57
# speedup vs baseline: 1.0123x; 1.0123x over previous
"""AttentionRNNCell Trainium2 kernel (v2: dual-layout fp8, no on-chip transposes).

Math (per batch row b):
  et[t]  = V_a . tanh( (h W_a + b_a) + x[t] U_a )        t in [0, TE)
  at     = exp(et);  s = sum(at)
  ctx    = (sum_t at[t] x[t]) / s
  zt     = sigmoid(h W_z + [inp, ctx] C_z + b_z)
  rt     = sigmoid(h W_r + [inp, ctx] C_r + b_r)
  tht    = tanh((rt*h) U_p + [inp, ctx] C_p + b_p)
  ht     = (1-zt)*h + zt*tht

Distribution: data-parallel over batch B=128 across 8 cores (16 rows each).

Design notes (per core):
  - The host ships x twice in fp8e4m3 (8.4MB each = 16.8MB total, same HBM
    bytes as one bf16 copy): xt = x^T in DoubleRow pair layout [p, j, t]
    (e = j*128+p) for the attention matmul, and xn natural [p, tc, e]
    (t = tc*128+p) for the context matmul. No on-chip transposes (the v1
    baseline spent ~650us in 512 xbar-transpose DMAs).
  - Attention matmul: U_a stationary [128,2,128] fp8 DoubleRow -> K=256 in
    one pass; out ux[u, t-chunk] in PSUM; ACT tanh (bias = (hW_a+b_a)^T col)
    writes fp8 in t'-interleaved order t' = tp*tc_n + tc so that:
  - V-dot: V_a stationary (M=1) streams tanh -> et row [1, te] in t' order;
    DVE copies rows to bf16 SBUF; one small DMA per b redistributes
    [1, te] -> [128, tc_n] (32B contiguous runs per partition).
  - exp on [128, tc_n] columns (accum_out -> partition-partial sums);
    s_b via ones-matmul; 1/s folded into the ctx-row copy (tensor_scalar).
  - Context matmul: at (fp8, DoubleRow pair layout) stationary, xn streams.
  - Gates in bf16, transposed orientation; sigmoid computed as
    0.5*tanh(0.5x)+0.5 to stay on the exp/tanh ACT table set.
"""

from contextlib import ExitStack

import numpy as np
import ml_dtypes

import concourse.bass as bass
import concourse.mybir as mybir
import concourse.tile as tile

F8NP = ml_dtypes.float8_e4m3
BF16 = ml_dtypes.bfloat16
F32 = mybir.dt.float32
BF = mybir.dt.bfloat16
F8 = mybir.dt.float8e4
AF = mybir.ActivationFunctionType
DR = mybir.MatmulPerfMode.DoubleRow

B, TE, U, IN_DIM = 128, 2048, 256, 256
N_CORES = 8
BS = B // N_CORES  # 16 batch rows per core
P = 128
UC = U // P  # u-chunks (2)


def split_multi_waits(nc, max_waits=1):
    """This container's walrus rejects instructions carrying more than one
    sync wait. Hoist extra waits onto standalone same-engine NoOps inserted
    immediately before the offending instruction (semantically identical:
    the engine blocks on each wait in order before executing it)."""
    n_new = 0
    for f in nc.m.functions:
        for blk in f.blocks:
            new_insts = []
            for inst in blk.instructions:
                si = inst.sync_info
                waits = list(si.on_wait) if si and si.on_wait else []
                if len(waits) > max_waits:
                    for w in waits[:-max_waits]:
                        nop = mybir.InstNoOp(
                            name=f"{inst.name}-hw{n_new}", ins=[], outs=[]
                        )
                        nop.engine = inst.engine
                        nop.sync_info = mybir.SyncInfo(on_wait=[w], on_update=[])
                        new_insts.append(nop)
                        n_new += 1
                    si.on_wait = waits[-max_waits:]
                new_insts.append(inst)
            blk.instructions = new_insts
    return n_new


def build_nc(bs=BS, te=TE, dr_main=True, dr_vdot=True, dr_ctx=True,
             swi_main=False, pair16=True, split_waits=True, debug=False):
    """Build the per-core Bass module. Parametrized so a small variant can be
    simulated quickly; the production shape is (bs=16, te=2048)."""
    tc_n = te // P           # 128-row t-chunks (16)
    nq = te // 512           # 512-wide et/vdot chunks (4)
    ux_w = min(1024, te)     # tanh tile width (PSUM: 2 banks)
    n_ux = te // ux_w        # tanh tiles per (b, uc)
    tcq = ux_w // P          # t-chunks per tanh tile (8)

    nc = bass.Bass()
    # xt[p, s, j] = x[t(s), j*128+p] with stream order t(s) = (s%tc_n)*128 + s//tc_n
    # (t' column order: ux/tanh/et come out so the per-b DRAM gather is clean;
    #  DoubleRow pair dim j innermost so the PE streams 2 adjacent fp8/cycle)
    if pair16:
        # DoubleRow pair at 16B stride: xt[p, g, j, ti] = x[t(g*16+ti), j*128+p]
        xt_d = nc.declare_dram_parameter("xt", [bs, P, te // 16, 2, 16], F8, isOutput=False)
        # xn[p, c, g, j, ei] = x[(2c+j)*128+p, g*16+ei]
        xn_d = nc.declare_dram_parameter(
            "xn", [bs, P, tc_n // 2, U // 16, 2, 16], F8, isOutput=False)
    else:
        xt_d = nc.declare_dram_parameter("xt", [bs, P, te, 2], F8, isOutput=False)
        # xn[p, c, e, j] = x[(2c+j)*128+p, e] (pair dim innermost)
        xn_d = nc.declare_dram_parameter("xn", [bs, P, tc_n // 2, U, 2], F8, isOutput=False)
    ua_d = nc.declare_dram_parameter("ua", [P, 2, U], F8, isOutput=False)
    # SwInterleave weight layout: [p, uc, 2*i+j] = U_a[j*128+p, uc*128 + (127-i)]
    uasw_d = nc.declare_dram_parameter("uasw", [P, UC, U], F8, isOutput=False)
    va_d = nc.declare_dram_parameter("va", [P, 2], BF, isOutput=False)
    wxpbT_d = nc.declare_dram_parameter("wxpbT", [P, UC, bs], F32, isOutput=False)
    hT_d = nc.declare_dram_parameter("hT", [P, UC, bs], F32, isOutput=False)
    g0T_d = nc.declare_dram_parameter("g0T", [3, P, UC, bs], F32, isOutput=False)
    cz_d = nc.declare_dram_parameter("cz", [P, UC, U], BF, isOutput=False)
    cr_d = nc.declare_dram_parameter("cr", [P, UC, U], BF, isOutput=False)
    cp_d = nc.declare_dram_parameter("cp", [P, UC, U], BF, isOutput=False)
    up_d = nc.declare_dram_parameter("up", [P, UC, U], BF, isOutput=False)
    id_d = nc.declare_dram_parameter("ident", [P, P], F32, isOutput=False)
    ht_d = nc.declare_dram_parameter("ht", [bs, U], F32, isOutput=True)
    # DRAM scratch for free-dim -> partition redistributions (SBUF APs cannot
    # express those; DRAM APs can be reshaped arbitrarily)
    etr_d = nc.declare_dram_parameter("etr_scratch", [bs, te], BF, isOutput=True)
    ctxr_d = nc.declare_dram_parameter("ctxr_scratch", [bs, U], BF, isOutput=True)
    if debug:
        dbg_tanh_d = nc.declare_dram_parameter("dbg_tanh", [P, UC, te], F8, isOutput=True)
        dbg_etrow_d = nc.declare_dram_parameter("dbg_etrow", [bs, te], BF, isOutput=True)
        dbg_etT_d = nc.declare_dram_parameter("dbg_etT", [bs, P, te // P], BF, isOutput=True)
        dbg_at_d = nc.declare_dram_parameter("dbg_at", [bs, P, 2, 16], F8, isOutput=True)
        dbg_spart_d = nc.declare_dram_parameter("dbg_spart", [P, bs], F32, isOutput=True)
        dbg_recip_d = nc.declare_dram_parameter("dbg_recip", [1, bs], F32, isOutput=True)
        dbg_ctxrows_d = nc.declare_dram_parameter("dbg_ctxrows", [bs, U], BF, isOutput=True)
        dbg_xn_d = nc.declare_dram_parameter("dbg_xn", [P, te // P // 2, U, 2], F8, isOutput=True)
        dbg_cps_d = nc.declare_dram_parameter("dbg_cps", [bs, U], F32, isOutput=True)

    with tile.TileContext(nc) as tc, ExitStack() as ctx:
        singles = ctx.enter_context(tc.tile_pool(name="singles", bufs=1))
        xt_p = ctx.enter_context(tc.tile_pool(name="xt", bufs=4))
        xn_p = ctx.enter_context(tc.tile_pool(name="xn", bufs=6))
        tanh_p = ctx.enter_context(tc.tile_pool(name="tanh", bufs=3))
        etrow_p = ctx.enter_context(tc.tile_pool(name="etrow", bufs=4))
        etT_p = ctx.enter_context(tc.tile_pool(name="etT", bufs=4))
        at_p = ctx.enter_context(tc.tile_pool(name="at", bufs=4))
        small_p = ctx.enter_context(tc.tile_pool(name="small", bufs=4))
        ux_ps = ctx.enter_context(tc.tile_pool(name="uxps", bufs=2, space="PSUM"))
        et_ps = ctx.enter_context(tc.tile_pool(name="etps", bufs=2, space="PSUM"))
        ctx_ps = ctx.enter_context(tc.tile_pool(name="ctxps", bufs=2, space="PSUM"))

        # first-matmul weights (tiny) first, then the first row's xt
        ua_sb = singles.tile([P, 2, U], F8)
        if swi_main:
            nc.sync.dma_start(out=ua_sb, in_=uasw_d[:, :, :])
        else:
            nc.sync.dma_start(out=ua_sb, in_=ua_d[:, :, :])
        wxpb_pre = singles.tile([P, UC, bs], F32, name="wxpb_sb")
        nc.scalar.dma_start(out=wxpb_pre, in_=wxpbT_d[:, :, :])
        xt_shape0 = [P, te // 16, 2, 16] if pair16 else [P, te, 2]
        xt_first = xt_p.tile(xt_shape0, F8, name="xtpre0")
        nc.sync.dma_start(out=xt_first, in_=xt_d[0])

        # prefetch the first batch rows' x
        xt_shape = [P, te // 16, 2, 16] if pair16 else [P, te, 2]
        xn_shape = ([P, tc_n // 2, U // 16, 2, 16] if pair16 else
                    [P, tc_n // 2, U, 2])
        xt_pre, xn_pre = {}, {}
        xt_pre[0] = xt_first
        for b in range(1, min(2, bs)):
            xt_pre[b] = xt_p.tile(xt_shape, F8, name=f"xtpre{b}")
            nc.sync.dma_start(out=xt_pre[b], in_=xt_d[b])
        for b in range(min(2, bs)):
            xn_pre[b] = xn_p.tile(xn_shape, F8, name=f"xnpre{b}")
            nc.scalar.dma_start(out=xn_pre[b], in_=xn_d[b])
        va_sb = singles.tile([P, 2], BF)
        nc.scalar.dma_start(out=va_sb, in_=va_d[:, :])
        wxpb_sb = wxpb_pre
        hT_sb = singles.tile([P, UC, bs], F32)
        nc.scalar.dma_start(out=hT_sb, in_=hT_d[:, :, :])
        g0_sb = singles.tile([P, 3, UC, bs], F32)
        nc.scalar.dma_start(out=g0_sb, in_=g0T_d[:, :, :, :].rearrange("g p c b -> p g c b"))
        gate_w = {}
        for name, d in (("cz", cz_d), ("cr", cr_d), ("cp", cp_d), ("up", up_d)):
            w_sb = singles.tile([P, UC, U], BF, name=f"{name}_sb")
            nc.scalar.dma_start(out=w_sb, in_=d[:, :, :])
            gate_w[name] = w_sb
        id_sb = singles.tile([P, P], F32)
        nc.scalar.dma_start(out=id_sb, in_=id_d[:, :])
        ones_sb = singles.tile([P, 1], F32)
        nc.vector.memset(ones_sb, 1.0)
        ones_row = singles.tile([1, P], F32)
        nc.vector.memset(ones_row, 1.0)
        s_part = singles.tile([P, bs], F32)
        recip_row = singles.tile([1, 16], F32)
        nc.vector.memset(recip_row, 1.0)
        ctx_rows = singles.tile([1, bs, U], BF)

        def finish_row(b, etT, xn_sb):
            # exp -> at (fp8, DoubleRow pair layout [p, j, tc2]); partial sums
            at8 = at_p.tile([P, 2, 16], F8)
            nc.scalar.activation(
                out=at8[:, :, 0 : tc_n // 2].rearrange("p j c -> p c j"),
                in_=etT,
                func=AF.Exp,
                accum_out=s_part[:, b : b + 1],
            )

            # s_b = sum over partitions; 1/s applied after the ctx transpose
            # (lives in the ctx pool: in the et pool its slow consumer -- the
            # DVE reciprocal -- held a buffer the next vdot needed)
            s_ps = ctx_ps.tile([1, 1], F32, tag="ctxps", name="s_ps")
            nc.tensor.matmul(out=s_ps, lhsT=ones_sb, rhs=s_part[:, b : b + 1])
            nc.vector.reciprocal(recip_row[:, b : b + 1], s_ps)

            # context matmul: at stationary, xn streams; ctx row [1, U]
            cps = ctx_ps.tile([1, U], F32, tag="ctxps")
            if dr_ctx:
                for j in range(tc_n // 2):
                    nc.tensor.matmul(
                        out=cps,
                        lhsT=at8[:, :, j : j + 1],
                        rhs=(xn_sb[:, j, :, :, :].rearrange("p g k e -> p k g e")
                             if pair16 else
                             xn_sb[:, j, :, :].rearrange("p e j -> p j e")),
                        perf_mode=DR,
                        start=(j == 0),
                        stop=(j == tc_n // 2 - 1),
                    )
            else:
                for tcc in range(tc_n):
                    nc.tensor.matmul(
                        out=cps,
                        lhsT=at8[:, tcc % 2, tcc // 2 : tcc // 2 + 1],
                        rhs=(xn_sb[:, tcc // 2, :, tcc % 2, :]
                             if pair16 else xn_sb[:, tcc // 2, :, tcc % 2]),
                        start=(tcc == 0),
                        stop=(tcc == tc_n - 1),
                    )
            nc.vector.tensor_copy(ctx_rows[:, b, :], cps)
            nc.sync.dma_start(out=ctxr_d[b : b + 1, :], in_=ctx_rows[:, b, :])
            if debug:
                cps_tap = small_p.tile([1, U], F32, tag="cpstap")
                nc.vector.tensor_copy(cps_tap, cps)
                nc.sync.dma_start(out=dbg_cps_d[b : b + 1, :], in_=cps_tap)
                nc.sync.dma_start(out=dbg_at_d[b, :, :, :], in_=at8)

        def mm_half(b, uc, xt_sb, tanh_t):
            """Attention matmul for one u-chunk over the full t range, in
            [128, ux_w] PSUM tiles each followed by one tanh."""
            for c in range(n_ux):
                ux = ux_ps.tile([P, ux_w], F32, tag="uxps", name=f"ux{uc}_{c}")
                for m0 in range(0, ux_w, 512):
                    t0 = c * ux_w + m0
                    if pair16:
                        rhs_mm = xt_sb[:, t0 // 16 : t0 // 16 + 32, :, :].rearrange(
                            "p g j t -> p j g t")
                    else:
                        rhs_mm = xt_sb[:, t0 : t0 + 512, :].rearrange("p t j -> p j t")
                    if swi_main:
                        nc.tensor.matmul(
                            out=ux[:, m0 : m0 + 512],
                            lhsT=ua_sb[:, uc, :],
                            rhs=rhs_mm,
                            perf_mode=mybir.MatmulPerfMode.DoubleRowSwInterleave,
                        )
                    elif dr_main:
                        nc.tensor.matmul(
                            out=ux[:, m0 : m0 + 512],
                            lhsT=ua_sb[:, :, uc * P : (uc + 1) * P],
                            rhs=rhs_mm,
                            perf_mode=DR,
                        )
                    else:
                        for j in range(2):
                            nc.tensor.matmul(
                                out=ux[:, m0 : m0 + 512],
                                lhsT=ua_sb[:, j, uc * P : (uc + 1) * P],
                                rhs=(xt_sb[:, t0 // 16 : t0 // 16 + 32, j, :]
                                     if pair16 else xt_sb[:, t0 : t0 + 512, j]),
                                start=(j == 0),
                                stop=(j == 1),
                            )
                nc.scalar.activation(
                    out=tanh_t[:, uc, c * ux_w : (c + 1) * ux_w],
                    in_=ux,
                    func=AF.Tanh,
                    bias=wxpb_sb[:, uc, b : b + 1],
                )

        def vdot_row(b, tanh_t, xn_sb):
            """V-dot -> et row (t' order), DVE copy to bf16, DRAM round trip."""
            et_row = etrow_p.tile([1, te], BF)
            for q in range(nq):
                etq = et_ps.tile([1, 512], F32, tag="etps", name=f"etq{q}")
                for uc in range(UC):
                    nc.tensor.matmul(
                        out=etq,
                        lhsT=va_sb[:, uc : uc + 1],
                        rhs=tanh_t[:, uc, q * 512 : (q + 1) * 512],
                        start=(uc == 0),
                        stop=(uc == UC - 1),
                    )
                nc.vector.tensor_copy(et_row[:, q * 512 : (q + 1) * 512], etq)
            nc.sync.dma_start(out=etr_d[b : b + 1, :], in_=et_row)
            etT = etT_p.tile([P, tc_n], BF)
            nc.sync.dma_start(
                out=etT, in_=etr_d[b, :].rearrange("(tp tc) -> tp tc", tc=tc_n)
            )
            if debug:
                if b == 0:
                    nc.sync.dma_start(out=dbg_tanh_d[:, :, :], in_=tanh_t)
                    if not pair16:
                        nc.sync.dma_start(out=dbg_xn_d[:, :, :], in_=xn_sb)
                nc.sync.dma_start(out=dbg_etrow_d[b : b + 1, :], in_=et_row)
                nc.sync.dma_start(out=dbg_etT_d[b, :, :], in_=etT)
            return etT

        # ---- streaming loop, 2-deep software pipeline ----
        # iter i: load(i), mm(uc0,i), vdot(i-1), mm(uc1,i), exp/s/ctx(i-2).
        # The in-order PE never waits: vdot(i-1) fills the tanh(uc0,i) window,
        # and the et DRAM round trip for row i-1 has a full iteration to land
        # before exp(i-1) runs in iter i+1.
        loaded = dict(xt_pre)
        for b in xt_pre:
            loaded[b] = (xt_pre[b], xn_pre[b])
        tiles = {}
        pend = []
        for i in range(bs + 2):
            if i + 2 < bs and (i + 2) not in loaded:
                xt_sb = xt_p.tile(xt_shape, F8)  # [e%128, t' order, j]
                nc.sync.dma_start(out=xt_sb, in_=xt_d[i + 2])
                xn_sb = xn_p.tile(xn_shape, F8)
                nc.scalar.dma_start(out=xn_sb, in_=xn_d[i + 2])
                loaded[i + 2] = (xt_sb, xn_sb)
            if i < bs:
                xt_sb, xn_sb = loaded.pop(i)
                tanh_t = tanh_p.tile([P, UC, te], BF)  # [u%128, uc, t']
                tiles[i] = (xt_sb, xn_sb, tanh_t)
                mm_half(i, 0, xt_sb, tanh_t)
            if 0 <= i - 1 < bs:
                bp = i - 1
                etT = vdot_row(bp, tiles[bp][2], tiles[bp][1])
                pend.append((bp, etT, tiles[bp][1]))
            if i < bs:
                mm_half(i, 1, xt_sb, tanh_t)
            if pend and pend[0][0] == i - 2:
                finish_row(*pend.pop(0))

        if debug:
            nc.sync.dma_start(out=dbg_spart_d[:, :], in_=s_part)
            nc.sync.dma_start(out=dbg_recip_d[:, :], in_=recip_row)
            nc.sync.dma_start(out=dbg_ctxrows_d[:, :], in_=ctx_rows[0, :, :])

        # ---- tail: transpose ctx rows (straight from DRAM), gates ----
        ctxTu = small_p.tile([P, UC, 16], BF, name="ctxTu")
        if bs < 16:
            nc.vector.memset(ctxTu, 0.0)
        for e in range(UC):
            nc.sync.dma_start_transpose(
                out=ctxTu[:, e, 0:bs], in_=ctxr_d[:, e * P : (e + 1) * P]
            )
        # normalize: ctxT = ctxT_unnorm * (1/s), broadcast 1/s to all partitions
        bc_ps = et_ps.tile([P, 16], F32, tag="etps", name="bc_ps")
        nc.tensor.matmul(out=bc_ps, lhsT=ones_row, rhs=recip_row)
        recip128 = small_p.tile([P, 16], F32, name="recip128")
        nc.vector.tensor_copy(recip128, bc_ps)
        ctxT = small_p.tile([P, UC, 16], BF, name="ctxT")
        for e in range(UC):
            nc.vector.tensor_mul(ctxT[:, e, :], ctxTu[:, e, :], recip128)

        def gate_psum(w_names_rhs, name):
            """psum[uc] = sum over (w, rhs) pairs of w^T @ rhs, per u-chunk."""
            outs = []
            for uc in range(UC):
                g = et_ps.tile([P, bs], F32, tag="etps", name=f"{name}{uc}")
                n_mm = sum(UC for _ in w_names_rhs)
                i = 0
                for w_sb, rhs_fn in w_names_rhs:
                    for e in range(UC):
                        nc.tensor.matmul(
                            out=g,
                            lhsT=w_sb[:, e, uc * P : (uc + 1) * P],
                            rhs=rhs_fn(e),
                            start=(i == 0),
                            stop=(i == n_mm - 1),
                        )
                        i += 1
                outs.append(g)
            return outs

        # zt^T, rt^T = sigmoid(g0 + C_*^T ctx^T) ; sigmoid via tanh:
        # sigmoid(x) = 0.5*tanh(0.5x) + 0.5 (stays on the exp/tanh table set)
        zt_sb = small_p.tile([P, UC, bs], F32)
        rt_sb = small_p.tile([P, UC, bs], F32)
        for gi, (wname, dst) in enumerate((("cz", zt_sb), ("cr", rt_sb))):
            gps = gate_psum([(gate_w[wname], lambda e: ctxT[:, e, 0:bs])], wname)
            for uc in range(UC):
                tmp = small_p.tile([P, bs], F32, tag="gtmp", name=f"t{wname}{uc}")
                nc.vector.tensor_add(tmp, gps[uc], g0_sb[:, gi, uc, :])
                th = small_p.tile([P, bs], F32, tag="gtmp", name=f"th{wname}{uc}")
                nc.scalar.activation(out=th, in_=tmp, func=AF.Tanh, scale=0.5)
                nc.vector.tensor_scalar(
                    dst[:, uc, :], th, 0.5, 0.5,
                    mybir.AluOpType.mult, mybir.AluOpType.add,
                )

        # rh^T = rt^T * h^T ; tht^T = tanh(g0p + U_p^T rh^T + C_p^T ctx^T)
        rh_sb = small_p.tile([P, UC, bs], BF)
        for uc in range(UC):
            nc.vector.tensor_mul(rh_sb[:, uc, :], rt_sb[:, uc, :], hT_sb[:, uc, :])
        gps = gate_psum(
            [(gate_w["up"], lambda e: rh_sb[:, e, :]),
             (gate_w["cp"], lambda e: ctxT[:, e, 0:bs])],
            "cp",
        )
        ht_nat = small_p.tile([bs, U], F32)
        for uc in range(UC):
            tmp = small_p.tile([P, bs], F32, tag="gtmp", name=f"tp{uc}")
            nc.vector.tensor_add(tmp, gps[uc], g0_sb[:, 2, uc, :])
            tht = small_p.tile([P, bs], F32, tag="gtmp", name=f"tht{uc}")
            nc.scalar.activation(out=tht, in_=tmp, func=AF.Tanh)
            # ht^T = h^T + zt^T*(tht^T - h^T)
            nc.vector.tensor_sub(tht, tht, hT_sb[:, uc, :])
            nc.vector.tensor_mul(tht, tht, zt_sb[:, uc, :])
            nc.vector.tensor_add(tht, tht, hT_sb[:, uc, :])
            tp = et_ps.tile([bs, P], F32, tag="etps", name=f"htp{uc}")
            nc.tensor.transpose(tp, tht[:, 0:bs], id_sb)
            nc.vector.tensor_copy(ht_nat[:, uc * P : (uc + 1) * P], tp)
        nc.sync.dma_start(out=ht_d[:, :], in_=ht_nat)

    if split_waits:
        split_multi_waits(nc)
    return nc


def _host_prep(inputs, h_tm, V_a, W_a, U_a, b_a, C_z, W_z, b_z, C_r, W_r, b_r,
               C_p, U_p, b_p):
    """Fold everything not depending on x_seq into small per-core tensors."""
    wxpb = h_tm @ W_a + b_a                                # [B, U]
    g_z0 = h_tm @ W_z + inputs @ C_z[:IN_DIM] + b_z        # [B, U]
    g_r0 = h_tm @ W_r + inputs @ C_r[:IN_DIM] + b_r
    g_p0 = inputs @ C_p[:IN_DIM] + b_p

    def chunkT(a):  # [bs, U] -> [P, UC, bs] with [p, c, b] = a[b, c*128+p]
        return np.ascontiguousarray(
            a.T.reshape(UC, P, -1).transpose(1, 0, 2).astype(np.float32)
        )

    def wprep(w):  # [U, U] -> [P, UC, U] bf16 with [p, c, u] = w[c*128+p, u]
        return np.ascontiguousarray(
            w.reshape(UC, P, U).transpose(1, 0, 2).astype(BF16)
        )

    # SwInterleave weights: [p, uc, 2*i+j] = U_a[j*128+p, uc*128 + (127-i)]
    ua_pj = U_a.reshape(2, P, U).transpose(1, 0, 2)   # [p, j, u]
    uasw = np.zeros((P, UC, U), dtype=np.float32)
    i_idx = np.arange(P)
    for uc_i in range(UC):
        cols = uc_i * P + (P - 1 - i_idx)             # reversed column order
        for j in range(2):
            uasw[:, uc_i, 2 * i_idx + j] = ua_pj[:, j, cols]
    shared = {
        "ua": np.ascontiguousarray(
            U_a.reshape(2, P, U).transpose(1, 0, 2).astype(F8NP)
        ),
        "uasw": np.ascontiguousarray(uasw.astype(F8NP)),
        "va": np.ascontiguousarray(V_a.reshape(2, P).T.astype(BF16)),
        "cz": wprep(C_z[IN_DIM:]),
        "cr": wprep(C_r[IN_DIM:]),
        "cp": wprep(C_p[IN_DIM:]),
        "up": wprep(U_p),
        "ident": np.eye(P, dtype=np.float32),
    }
    per_core = []
    for c in range(N_CORES):
        s = slice(c * BS, (c + 1) * BS)
        per_core.append(
            {
                "wxpbT": chunkT(wxpb[s]),
                "hT": chunkT(h_tm[s]),
                "g0T": np.ascontiguousarray(
                    np.stack([chunkT(g_z0[s]), chunkT(g_r0[s]), chunkT(g_p0[s])])
                ),
                **shared,
            }
        )
    return per_core


def _x_prep(x_seq, pair16=True):
    """Per-core fp8 dual layouts of x (see build_nc layout comments)."""
    x8 = x_seq.astype(F8NP)
    tc_n = TE // P
    out = []
    for c in range(N_CORES):
        xs = x8[c * BS : (c + 1) * BS]                     # [bs, TE, U]
        # xt[b, p, s, j] = x[b, t(s), j*128+p], t(s) = (s%tc_n)*128 + s//tc_n
        xp = xs.reshape(BS, tc_n, P, U).transpose(0, 2, 1, 3).reshape(BS, TE, U)
        xt = np.ascontiguousarray(
            xp.reshape(BS, TE, 2, P).transpose(0, 3, 1, 2)
        )
        # xn[b, p, c, e, j] = x[b, (2c+j)*128+p, e]
        xn = np.ascontiguousarray(
            xs.reshape(BS, tc_n // 2, 2, P, U).transpose(0, 3, 1, 4, 2)
        )
        if pair16:
            xt = np.ascontiguousarray(
                xt.reshape(BS, P, TE // 16, 16, 2).transpose(0, 1, 2, 4, 3)
            )
            xn = np.ascontiguousarray(
                xn.reshape(BS, P, tc_n // 2, U // 16, 16, 2).transpose(0, 1, 2, 3, 5, 4)
            )
        out.append({"xt": xt, "xn": xn})
    return out


def kernel(inputs, h_tm, x_seq, V_a, W_a, U_a, b_a, C_z, W_z, b_z,
           C_r, W_r, b_r, C_p, U_p, b_p):
    from concourse.bass_utils import run_bass_kernel_spmd

    args = {k: np.asarray(v, dtype=np.float32) for k, v in dict(
        inputs=inputs, h_tm=h_tm, V_a=V_a, W_a=W_a, U_a=U_a, b_a=b_a,
        C_z=C_z, W_z=W_z, b_z=b_z, C_r=C_r, W_r=W_r, b_r=b_r,
        C_p=C_p, U_p=U_p, b_p=b_p).items()}
    x_seq = np.asarray(x_seq, dtype=np.float32)

    per_core = _host_prep(**args)
    x_maps = _x_prep(x_seq)
    in_maps = [{**per_core[c], **x_maps[c]} for c in range(N_CORES)]

    nc = build_nc()
    res = run_bass_kernel_spmd(nc, in_maps, core_ids=list(range(N_CORES)))
    return np.concatenate([res.results[c]["ht"] for c in range(N_CORES)], axis=0)
